# revision 3
# baseline (speedup 1.0000x reference)
"""Trainium2 Bass kernel for nn_Actor_IntentionEncoder (gnn_message_passing).

Data-parallel over N = B*A = 8192 rows; core c owns rows [1024c, 1024c+1024)
= output groups [64c, 64c+64).

Phase 1 (hypernet) runs in transposed layout: partitions p = (d, s) with
d in [0,64), s in {0,1}; free axis = local row n. For each h'-pair
j in [0,64):
    wps[p, n] = sum_k hw2B[k, 128j+p] * hT[k, n]          (PE, [128,1024])
    ACT path: t = relu(wps + b2B[:, j])  (per-partition bias, free)
              m = t * xx                 (DVE tensor_tensor f16)
    DVE path: m = max(wps, -b2B[:, j]) * xx   (fused STT; relu(w+b) =
              max(w,-b)+b, the +x*b term is one K=64 corr matmul)
    emb_pre^T[2j+s, n] += sum_d m[d+64s, n]  (PE selector matmul, sliding
              window slice of a [128,254] 0/1 matrix, PSUM accumulate)
emb^T = tanh(emb_pre^T) (ACT). Group means via DVE strided reduce ->
AllGather (16KB) -> phase 2 (value MLP + attention) in the baseline's
mixed layout, vals batched 4 chunks per PSUM bank.
"""

import sys

sys.path.insert(0, "/opt/trn_rl_repo")

import ml_dtypes
import numpy as np

import concourse.bacc as bacc
import concourse.tile as tile
from concourse import mybir
from concourse.bass_utils import run_bass_kernel_spmd

F32 = mybir.dt.float32
F16 = mybir.dt.float16
AF = mybir.ActivationFunctionType
OP = mybir.AluOpType
NPF16 = np.float16

NCORES = 8
B, A, DS, DO, H = 512, 16, 32, 32, 128
D = DS + DO  # 64
N = B * A  # 8192
RPC = N // NCORES  # rows per core = 1024
GPC = B // NCORES  # groups per core = 64
NJ = 64  # h'-pairs
SEL_LAG = 3
DEBUG = False

# j-pairs whose evacuation goes through ACT (relu + per-partition bias);
# the rest go through the DVE fused max-mult path.
ACT_J = [j for j in range(NJ) if (j % 16) < 11]
ACT_JSET = set(ACT_J)

_CACHE = {}


def _build():
    nc = bacc.Bacc("TRN2", target_bir_lowering=False, debug=False,
                   num_devices=NCORES)

    def inp(name, shape, dt=F32):
        return nc.dram_tensor(name, list(shape), dt, kind="ExternalInput").ap()

    xxT_d = inp("xxT", (128, RPC), F16)
    hw1_d = inp("hw1", (D, H), F16)
    hb1_d = inp("hb1", (H, 1))
    hw2B_d = inp("hw2B", (H, D * H), F16)
    b2B_d = inp("b2B", (128, NJ))
    negb2B_d = inp("negb2B", (128, NJ))
    bcorr_d = inp("bcorr", (D, H), F16)
    selpad_d = inp("selpad", (128, 126), F16)
    vw1_d = inp("vw1", (H, H), F16)
    vb1_d = inp("vb1", (H, 1))
    vw2_d = inp("vw2", (H, H), F16)
    vb2row_d = inp("vb2row", (1, H), F16)
    onescol_d = inp("onescol", (1, H), F16)
    ones128_d = inp("ones128", (H, 1), F16)
    aw1a_d = inp("aw1a", (H, H), F16)
    aw1b_d = inp("aw1b", (H, H), F16)  # pre-scaled by 1/A
    ab1_d = inp("ab1", (H, 1))
    aw2_d = inp("aw2", (H, H), F16)
    ab2_d = inp("ab2", (H, 1))
    aw3_d = inp("aw3", (H, 1), F16)
    ab3_d = inp("ab3", (128, 1))
    pmask_d = inp("pmask", (128, 8), F16)

    out_d = nc.dram_tensor("out", [GPC, H], F32, kind="ExternalOutput").ap()
    if DEBUG:
        dbg_hT = nc.dram_tensor("dbg_hT", [H, RPC], F16,
                                kind="ExternalOutput").ap()
        dbg_emb = nc.dram_tensor("dbg_emb", [128, RPC], F16,
                                 kind="ExternalOutput").ap()
        dbg_mean = nc.dram_tensor("dbg_mean", [128, B], F16,
                                  kind="ExternalOutput").ap()
        dbg_exp = nc.dram_tensor("dbg_exp", [128, 8], F32,
                                 kind="ExternalOutput").ap()
        dbg_y1T = nc.dram_tensor("dbg_y1T", [128, RPC], F16,
                                 kind="ExternalOutput").ap()
        dbg_mean2 = nc.dram_tensor("dbg_mean2", [128, B], F16,
                                   kind="ExternalOutput").ap()

    with tile.TileContext(nc) as tc:
        with (
            tc.tile_pool(name="const", bufs=1) as constp,
            tc.tile_pool(name="keep", bufs=1) as keepp,
            tc.tile_pool(name="work", bufs=4) as workp,
            tc.tile_pool(name="wps", bufs=3, space="PSUM") as wpsp,
            tc.tile_pool(name="embps", bufs=1, space="PSUM") as embpsp,
            tc.tile_pool(name="dram", bufs=1, space="DRAM") as dramp,
        ):
            def load(ap, shape, name, dt=F32):
                t = constp.tile(list(shape), dt, tag=name, name=name + "_sb")
                nc.sync.dma_start(t[:], ap[:])
                return t

            # ---- loads: phase-1 essentials first ----
            xxT_sb = load(xxT_d, (128, RPC), "xxT", F16)
            hw1_sb = load(hw1_d, (D, H), "hw1", F16)
            hb1_sb = load(hb1_d, (H, 1), "hb1")
            selpad_sb = load(selpad_d, (128, 126), "selpad", F16)
            b2B_sb = load(b2B_d, (128, NJ), "b2B")
            negb2B_sb = load(negb2B_d, (128, NJ), "negb2B")
            bcorr_sb = load(bcorr_d, (D, H), "bcorr", F16)
            hw2B_sb = constp.tile([H, D * H], F16, tag="hw2B")
            for s in range(8):
                nc.sync.dma_start(hw2B_sb[:, s * 1024:(s + 1) * 1024],
                                  hw2B_d[:, s * 1024:(s + 1) * 1024])
            vw1_sb = load(vw1_d, (H, H), "vw1", F16)
            vb1_sb = load(vb1_d, (H, 1), "vb1")
            vw2_sb = load(vw2_d, (H, H), "vw2", F16)
            vb2r_sb = load(vb2row_d, (1, H), "vb2r", F16)
            onescol_sb = load(onescol_d, (1, H), "onescol", F16)
            ones128_sb = load(ones128_d, (H, 1), "ones128", F16)
            aw1a_sb = load(aw1a_d, (H, H), "aw1a", F16)
            aw1b_sb = load(aw1b_d, (H, H), "aw1b", F16)
            ab1_sb = load(ab1_d, (H, 1), "ab1")
            aw2_sb = load(aw2_d, (H, H), "aw2", F16)
            ab2_sb = load(ab2_d, (H, 1), "ab2")
            aw3_sb = load(aw3_d, (H, 1), "aw3", F16)
            ab3_sb = load(ab3_d, (128, 1), "ab3")
            pmask_sb = load(pmask_d, (128, 8), "pmask", F16)

            # ---- hT = relu(hw1^T @ x^T + hb1)  [128, 1024] f16 ----
            hps = wpsp.tile([128, RPC], F32, tag="wps", name="hps")
            for h in range(2):
                nc.tensor.matmul(hps[:, h * 512:(h + 1) * 512], hw1_sb[:],
                                 xxT_sb[0:D, h * 512:(h + 1) * 512],
                                 start=True, stop=True)
            hT_sb = keepp.tile([H, RPC], F16, tag="hT")
            nc.scalar.activation(hT_sb[:], hps[:], AF.Relu, bias=hb1_sb[:])

            # ---- phase 1: hypernet j-loop ----
            embps = embpsp.tile([128, RPC], F32, tag="embps")
            for h in range(2):
                nc.tensor.matmul(embps[:, h * 512:(h + 1) * 512], bcorr_sb[:],
                                 xxT_sb[0:D, h * 512:(h + 1) * 512],
                                 start=True, stop=True)

            m_tiles = [None] * NJ

            def emit_hyper(j):
                wps = wpsp.tile([128, RPC], F32, tag="wps", name=f"wps_{j}")
                for h in range(2):
                    nc.tensor.matmul(wps[:, h * 512:(h + 1) * 512],
                                     hw2B_sb[:, j * 128:(j + 1) * 128],
                                     hT_sb[:, h * 512:(h + 1) * 512],
                                     start=True, stop=True)
                m = workp.tile([128, RPC], F16, tag="m", name=f"m_{j}")
                if j in ACT_JSET:
                    t = workp.tile([128, RPC], F16, tag="t", name=f"t_{j}")
                    nc.scalar.activation(t[:], wps[:], AF.Relu,
                                         bias=b2B_sb[:, j:j + 1])
                    nc.vector.tensor_tensor(m[:], t[:], xxT_sb[:], op=OP.mult)
                else:
                    nc.vector.scalar_tensor_tensor(
                        m[:], wps[:], negb2B_sb[:, j:j + 1], xxT_sb[:],
                        OP.max, OP.mult)
                m_tiles[j] = m

            def emit_selector(j):
                m = m_tiles[j]
                q = 64 * (j // 32)
                c0 = 62 - 2 * (j % 32)
                for h in range(2):
                    nc.tensor.matmul(
                        embps[q:q + 64, h * 512:(h + 1) * 512],
                        selpad_sb[:, c0:c0 + 64],
                        m[:, h * 512:(h + 1) * 512],
                        start=False, stop=True, skip_group_check=True)
                m_tiles[j] = None

            embT_sb = keepp.tile([128, RPC], F16, tag="embT")
            cc_in, cc_out = [], []
            for half in range(2):
                cc_in.append(dramp.tile([64, GPC], F16, tag=f"cc_in{half}",
                                        name=f"cc_in{half}"))
                cc_out.append(dramp.tile([NCORES * 64, GPC], F16,
                                         tag=f"cc_out{half}",
                                         name=f"cc_out{half}"))

            def emit_gather(half):
                rows = slice(64 * half, 64 * half + 64)
                nc.scalar.activation(embT_sb[rows, :], embps[rows, :], AF.Tanh)
                msum = workp.tile([64, GPC], F32, tag="msum",
                                  name=f"msum{half}")
                nc.vector.tensor_reduce(
                    msum[:],
                    embT_sb[rows, :].rearrange("p (g a) -> p g a", a=A),
                    axis=mybir.AxisListType.X, op=OP.add)
                msumh = workp.tile([64, GPC], F16, tag="msumh",
                                   name=f"msumh{half}")
                nc.scalar.copy(msumh[:], msum[:])
                nc.sync.dma_start(cc_in[half][:], msumh[:])
                nc.gpsimd.collective_compute(
                    "AllGather", OP.bypass,
                    replica_groups=[list(range(NCORES))],
                    ins=[cc_in[half].opt()], outs=[cc_out[half].opt()])

            for j in range(NJ + SEL_LAG):
                if j < NJ:
                    emit_hyper(j)
                if j >= SEL_LAG:
                    emit_selector(j - SEL_LAG)
                if j - SEL_LAG == 31:
                    emit_gather(0)
            emit_gather(1)
            if DEBUG:
                nc.sync.dma_start(dbg_hT[:], hT_sb[:])
                nc.sync.dma_start(dbg_emb[:], embT_sb[:])

            # ---- phase 2a (overlaps collective): value MLP + y1a ----
            def chsl(t, ch):
                return t[:, ch * 128:(ch + 1) * 128]

            v1ps = wpsp.tile([128, RPC], F32, tag="wps", name="v1ps")
            for ch in range(8):
                nc.tensor.matmul(chsl(v1ps, ch), vw1_sb[:], chsl(embT_sb, ch),
                                 start=True, stop=True)
            v1T = workp.tile([128, RPC], F16, tag="m", name="v1T")
            nc.scalar.activation(v1T[:], v1ps[:], AF.Relu, bias=vb1_sb[:])

            vps = wpsp.tile([128, RPC], F32, tag="wps", name="vps")
            for ch in range(8):
                nc.tensor.matmul(chsl(vps, ch), onescol_sb[:], vb2r_sb[:],
                                 start=True, stop=False)
                nc.tensor.matmul(chsl(vps, ch), chsl(v1T, ch), vw2_sb[:],
                                 start=False, stop=True)
            vals_sb = keepp.tile([128, RPC], F16, tag="vals")
            nc.scalar.activation(vals_sb[:], vps[:], AF.Relu)

            y1ps = embpsp.tile([128, RPC], F32, tag="embps", name="y1ps")

            # P_w tiles zeroed on gpsimd during the collective
            P_w = []
            for ch in range(8):
                pw = workp.tile([128, GPC], F16, tag=f"P_w{ch}",
                                name=f"P_w_{ch}")
                nc.gpsimd.memset(pw[:], 0.0)
                P_w.append(pw)

            meanT_full = keepp.tile([128, B], F16, tag="meanTf")
            for half in range(2):
                rows = slice(64 * half, 64 * half + 64)
                nc.sync.dma_start(
                    meanT_full[rows, :].rearrange("p (c g) -> p c g",
                                                  c=NCORES),
                    cc_out[half][:].rearrange("(c p) g -> p c g", c=NCORES))

            # ---- phase 2b: attention + weighted output ----
            if DEBUG:
                nc.sync.dma_start(dbg_mean2[:], meanT_full[:])
            for ch in range(8):
                mcol = 128 * (ch % 4)
                nc.tensor.matmul(chsl(y1ps, ch), aw1a_sb[:], chsl(embT_sb, ch),
                                 start=True, stop=False)
                nc.tensor.matmul(chsl(y1ps, ch), aw1b_sb[:],
                                 meanT_full[:, mcol:mcol + 128],
                                 start=False, stop=True)
            y1T = workp.tile([128, RPC], F16, tag="m", name="y1T")
            nc.scalar.activation(y1T[:], y1ps[:], AF.Relu, bias=ab1_sb[:])
            if DEBUG:
                nc.sync.dma_start(dbg_y1T[:], y1T[:])

            scps_t = wpsp.tile([128, RPC], F32, tag="wps", name="scps")
            so_t = wpsp.tile([128, RPC], F32, tag="wps", name="so")
            y2ps = wpsp.tile([128, RPC], F32, tag="wps", name="y2ps")
            for ch in range(8):
                nc.tensor.matmul(chsl(y2ps, ch), aw2_sb[:], chsl(y1T, ch),
                                 start=True, stop=True)
            y2T = workp.tile([128, RPC], F16, tag="m", name="y2T")
            nc.scalar.activation(y2T[:], y2ps[:], AF.Relu, bias=ab2_sb[:])
            for ch in range(8):
                nc.tensor.matmul(scps_t[:, ch:ch + 1], chsl(y2T, ch),
                                 aw3_sb[:], start=True, stop=True)
            exp_sb = workp.tile([128, 8], F32, tag="exp_sb")
            nc.scalar.activation(exp_sb[:], scps_t[:, 0:8], AF.Exp,
                                 bias=ab3_sb[:])
            if DEBUG:
                nc.sync.dma_start(dbg_mean[:], meanT_full[:])
                nc.sync.dma_start(dbg_exp[:], exp_sb[:])
            for ch in range(8):
                nc.gpsimd.tensor_scalar_mul(P_w[ch][:, ch * 8:(ch + 1) * 8],
                                            pmask_sb[:], exp_sb[:, ch:ch + 1])
                nc.tensor.matmul(so_t[0:GPC, 0:128], P_w[ch][:],
                                 chsl(vals_sb, ch),
                                 start=(ch == 0), stop=(ch == 7))
            for ch in range(8):
                nc.tensor.matmul(so_t[0:GPC, 128:129], P_w[ch][:],
                                 ones128_sb[:], start=(ch == 0), stop=(ch == 7))

            inv_S = workp.tile([GPC, 1], F32, tag="inv_S")
            nc.vector.reciprocal(inv_S[:], so_t[0:GPC, 128:129])
            out_sb = workp.tile([GPC, H], F32, tag="out_sb")
            nc.vector.tensor_scalar_mul(out_sb[:], so_t[0:GPC, 0:128],
                                        inv_S[:])
            nc.sync.dma_start(out_d[:], out_sb[:])

    nc.compile()
    return nc


def _prep_inputs(obs, latent, hw1, hb1, hw2, hb2, vw1, vb1, vw2, vb2,
                 aw1, ab1, aw2, ab2, aw3, ab3):
    f = np.float32
    fh = lambda a: np.ascontiguousarray(np.asarray(a, f).astype(NPF16))
    col = lambda a: np.ascontiguousarray(np.asarray(a, f).reshape(-1, 1))

    x_full = np.concatenate(
        [np.tile(obs, (A, 1)), latent.reshape(-1, DO)], axis=1).astype(f)

    # hw2B[k, 128j + 64s + d] = hw2[k, 128d + 2j + s]
    hw2B = np.asarray(hw2, f).reshape(H, D, NJ, 2).transpose(0, 2, 3, 1) \
        .reshape(H, D * H)
    # b2B[64s + d, j] = hb2[128d + 2j + s]
    b2B = np.asarray(hb2, f).reshape(D, NJ, 2).transpose(2, 0, 1) \
        .reshape(128, NJ)
    # corr term only for DVE (max-trick) j's
    bcorr = np.asarray(hb2, f).reshape(D, H).copy()
    for j in ACT_J:
        bcorr[:, 2 * j] = 0.0
        bcorr[:, 2 * j + 1] = 0.0
    selpad = np.zeros((128, 126), NPF16)
    for p in range(128):
        selpad[p, 62 + p // 64] = 1.0
    pmask = np.zeros((128, 8), NPF16)
    for r in range(128):
        pmask[r, r // 16] = 1.0

    shared = dict(
        hw1=fh(hw1), hb1=col(hb1),
        hw2B=fh(hw2B), b2B=np.ascontiguousarray(b2B),
        negb2B=np.ascontiguousarray(-b2B), bcorr=fh(bcorr),
        selpad=selpad,
        vw1=fh(vw1), vb1=col(vb1), vw2=fh(vw2),
        vb2row=fh(np.asarray(vb2).reshape(1, H)),
        onescol=np.ones((1, H), NPF16), ones128=np.ones((H, 1), NPF16),
        aw1a=fh(np.asarray(aw1)[:H]),
        aw1b=fh(np.asarray(aw1)[H:] / A),
        ab1=col(ab1), aw2=fh(aw2), ab2=col(ab2),
        aw3=fh(np.asarray(aw3).reshape(H, 1)),
        ab3=np.full((128, 1), np.float32(np.asarray(ab3).reshape(())), f),
        pmask=pmask,
    )
    in_maps = []
    for c in range(NCORES):
        xcT = x_full[c * RPC:(c + 1) * RPC].T  # [64, 1024]
        m = dict(shared)
        m["xxT"] = np.ascontiguousarray(
            np.concatenate([xcT, xcT], axis=0)).astype(NPF16)
        in_maps.append(m)
    return in_maps


def kernel(**inputs):
    obs = np.asarray(inputs["obs"], np.float32)
    latent = np.asarray(inputs["obs_intention_latent"], np.float32)
    in_maps = _prep_inputs(
        obs, latent, inputs["hw1"], inputs["hb1"], inputs["hw2"], inputs["hb2"],
        inputs["vw1"], inputs["vb1"], inputs["vw2"], inputs["vb2"],
        inputs["aw1"], inputs["ab1"], inputs["aw2"], inputs["ab2"],
        inputs["aw3"], inputs["ab3"])
    if "nc" not in _CACHE:
        _CACHE["nc"] = _build()
    res = run_bass_kernel_spmd(_CACHE["nc"], in_maps, list(range(NCORES)))
    _CACHE["res"] = res
    out = np.empty((B, H), np.float32)
    for c in range(NCORES):
        out[c * GPC:(c + 1) * GPC] = res.results[c]["out"]
    return out


if __name__ == "__main__":
    import reference
    inputs = reference.setup_inputs()
    inputs = {k: np.asarray(v) for k, v in inputs.items()}
    got = kernel(**inputs)
    exp = np.asarray(reference.reference(**reference.setup_inputs()))
    print("Relative error:", np.abs(got - exp).max() / (np.abs(exp).max() + 1e-9))


# revision 4
# speedup vs baseline: 1.0406x; 1.0406x over previous
"""Trainium2 Bass kernel for nn_Actor_IntentionEncoder (gnn_message_passing).

Data-parallel over N = B*A = 8192 rows; core c owns rows [1024c, 1024c+1024)
= output groups [64c, 64c+64).

Phase 1 (hypernet) runs in transposed layout: partitions p = (d, s) with
d in [0,64), s in {0,1}; free axis = local row n. For each h'-pair
j in [0,64):
    wps[p, n] = sum_k hw2B[k, 128j+p] * hT[k, n]          (PE, [128,1024])
    ACT path: t = relu(wps + b2B[:, j])  (per-partition bias, free)
              m = t * xx                 (DVE tensor_tensor f16)
    DVE path: m = max(wps, -b2B[:, j]) * xx   (fused STT; relu(w+b) =
              max(w,-b)+b, the +x*b term is one K=64 corr matmul)
    emb_pre^T[2j+s, n] += sum_d m[d+64s, n]  (PE selector matmul, sliding
              window slice of a [128,254] 0/1 matrix, PSUM accumulate)
emb^T = tanh(emb_pre^T) (ACT). Group means via DVE strided reduce ->
AllGather (16KB) -> phase 2 (value MLP + attention) in the baseline's
mixed layout, vals batched 4 chunks per PSUM bank.
"""

import sys

sys.path.insert(0, "/opt/trn_rl_repo")

import ml_dtypes
import numpy as np

import concourse.bacc as bacc
import concourse.tile as tile
from concourse import mybir
from concourse.bass_utils import run_bass_kernel_spmd

F32 = mybir.dt.float32
F16 = mybir.dt.float16
AF = mybir.ActivationFunctionType
OP = mybir.AluOpType
NPF16 = np.float16

NCORES = 8
B, A, DS, DO, H = 512, 16, 32, 32, 128
D = DS + DO  # 64
N = B * A  # 8192
RPC = N // NCORES  # rows per core = 1024
GPC = B // NCORES  # groups per core = 64
NJ = 64  # h'-pairs
SEL_LAG = 2
DEBUG = False

# j-pairs whose evacuation goes through ACT (relu + per-partition bias);
# the rest go through the DVE fused max-mult path.
ACT_J = [j for j in range(NJ) if (j % 16) < 11]
ACT_JSET = set(ACT_J)

_CACHE = {}


def _build():
    nc = bacc.Bacc("TRN2", target_bir_lowering=False, debug=False,
                   num_devices=NCORES)

    def inp(name, shape, dt=F32):
        return nc.dram_tensor(name, list(shape), dt, kind="ExternalInput").ap()

    xxT_d = inp("xxT", (128, RPC), F16)
    hw1_d = inp("hw1", (D, H), F16)
    hb1_d = inp("hb1", (H, 1))
    hw2B_d = inp("hw2B", (H, D * H), F16)
    b2B_d = inp("b2B", (128, NJ))
    negb2B_d = inp("negb2B", (128, NJ))
    bcorr_d = inp("bcorr", (D, H), F16)
    selpad_d = inp("selpad", (128, 126), F16)
    vw1_d = inp("vw1", (H, H), F16)
    vb1_d = inp("vb1", (H, 1))
    vw2_d = inp("vw2", (H, H), F16)
    vb2row_d = inp("vb2row", (1, H), F16)
    onescol_d = inp("onescol", (1, H), F16)
    ones128_d = inp("ones128", (H, 1), F16)
    aw1a_d = inp("aw1a", (H, H), F16)
    aw1b_d = inp("aw1b", (H, H), F16)  # pre-scaled by 1/A
    ab1_d = inp("ab1", (H, 1))
    aw2_d = inp("aw2", (H, H), F16)
    ab2_d = inp("ab2", (H, 1))
    aw3_d = inp("aw3", (H, 1), F16)
    ab3_d = inp("ab3", (128, 1))
    pmask_d = inp("pmask", (128, 8), F16)

    out_d = nc.dram_tensor("out", [GPC, H], F32, kind="ExternalOutput").ap()
    if DEBUG:
        dbg_hT = nc.dram_tensor("dbg_hT", [H, RPC], F16,
                                kind="ExternalOutput").ap()
        dbg_emb = nc.dram_tensor("dbg_emb", [128, RPC], F16,
                                 kind="ExternalOutput").ap()
        dbg_mean = nc.dram_tensor("dbg_mean", [128, B], F16,
                                  kind="ExternalOutput").ap()
        dbg_exp = nc.dram_tensor("dbg_exp", [128, 8], F32,
                                 kind="ExternalOutput").ap()
        dbg_y1T = nc.dram_tensor("dbg_y1T", [128, RPC], F16,
                                 kind="ExternalOutput").ap()
        dbg_mean2 = nc.dram_tensor("dbg_mean2", [128, B], F16,
                                   kind="ExternalOutput").ap()

    with tile.TileContext(nc) as tc:
        with (
            tc.tile_pool(name="const", bufs=1) as constp,
            tc.tile_pool(name="keep", bufs=1) as keepp,
            tc.tile_pool(name="work", bufs=4) as workp,
            tc.tile_pool(name="wps", bufs=3, space="PSUM") as wpsp,
            tc.tile_pool(name="embps", bufs=1, space="PSUM") as embpsp,
            tc.tile_pool(name="dram", bufs=1, space="DRAM") as dramp,
        ):
            def load(ap, shape, name, dt=F32):
                t = constp.tile(list(shape), dt, tag=name, name=name + "_sb")
                nc.sync.dma_start(t[:], ap[:])
                return t

            # ---- loads: phase-1 essentials first ----
            xxT_sb = load(xxT_d, (128, RPC), "xxT", F16)
            hw1_sb = load(hw1_d, (D, H), "hw1", F16)
            hb1_sb = load(hb1_d, (H, 1), "hb1")
            selpad_sb = load(selpad_d, (128, 126), "selpad", F16)
            b2B_sb = load(b2B_d, (128, NJ), "b2B")
            negb2B_sb = load(negb2B_d, (128, NJ), "negb2B")
            bcorr_sb = load(bcorr_d, (D, H), "bcorr", F16)
            hw2B_sb = constp.tile([H, D * H], F16, tag="hw2B")
            for s in range(8):
                nc.sync.dma_start(hw2B_sb[:, s * 1024:(s + 1) * 1024],
                                  hw2B_d[:, s * 1024:(s + 1) * 1024])
            vw1_sb = load(vw1_d, (H, H), "vw1", F16)
            vb1_sb = load(vb1_d, (H, 1), "vb1")
            vw2_sb = load(vw2_d, (H, H), "vw2", F16)
            vb2r_sb = load(vb2row_d, (1, H), "vb2r", F16)
            onescol_sb = load(onescol_d, (1, H), "onescol", F16)
            ones128_sb = load(ones128_d, (H, 1), "ones128", F16)
            aw1a_sb = load(aw1a_d, (H, H), "aw1a", F16)
            aw1b_sb = load(aw1b_d, (H, H), "aw1b", F16)
            ab1_sb = load(ab1_d, (H, 1), "ab1")
            aw2_sb = load(aw2_d, (H, H), "aw2", F16)
            ab2_sb = load(ab2_d, (H, 1), "ab2")
            aw3_sb = load(aw3_d, (H, 1), "aw3", F16)
            ab3_sb = load(ab3_d, (128, 1), "ab3")
            pmask_sb = load(pmask_d, (128, 8), "pmask", F16)

            # ---- hT = relu(hw1^T @ x^T + hb1)  [128, 1024] f16 ----
            hps = wpsp.tile([128, RPC], F32, tag="wps", name="hps")
            for h in range(2):
                nc.tensor.matmul(hps[:, h * 512:(h + 1) * 512], hw1_sb[:],
                                 xxT_sb[0:D, h * 512:(h + 1) * 512],
                                 start=True, stop=True)
            hT_sb = keepp.tile([H, RPC], F16, tag="hT")
            nc.scalar.activation(hT_sb[:], hps[:], AF.Relu, bias=hb1_sb[:])

            # ---- phase 1: hypernet j-loop ----
            embps = embpsp.tile([128, RPC], F32, tag="embps")
            for h in range(2):
                nc.tensor.matmul(embps[:, h * 512:(h + 1) * 512], bcorr_sb[:],
                                 xxT_sb[0:D, h * 512:(h + 1) * 512],
                                 start=True, stop=True)

            m_tiles = [None] * NJ

            def emit_hyper(j):
                wps = wpsp.tile([128, RPC], F32, tag="wps", name=f"wps_{j}")
                for h in range(2):
                    nc.tensor.matmul(wps[:, h * 512:(h + 1) * 512],
                                     hw2B_sb[:, j * 128:(j + 1) * 128],
                                     hT_sb[:, h * 512:(h + 1) * 512],
                                     start=True, stop=True)
                m = workp.tile([128, RPC], F16, tag="m", name=f"m_{j}")
                if j in ACT_JSET:
                    t = workp.tile([128, RPC], F16, tag="t", name=f"t_{j}")
                    nc.scalar.activation(t[:], wps[:], AF.Relu,
                                         bias=b2B_sb[:, j:j + 1])
                    nc.vector.tensor_tensor(m[:], t[:], xxT_sb[:], op=OP.mult)
                else:
                    nc.vector.scalar_tensor_tensor(
                        m[:], wps[:], negb2B_sb[:, j:j + 1], xxT_sb[:],
                        OP.max, OP.mult)
                m_tiles[j] = m

            def emit_selector(j):
                m = m_tiles[j]
                q = 64 * (j // 32)
                c0 = 62 - 2 * (j % 32)
                for h in range(2):
                    nc.tensor.matmul(
                        embps[q:q + 64, h * 512:(h + 1) * 512],
                        selpad_sb[:, c0:c0 + 64],
                        m[:, h * 512:(h + 1) * 512],
                        start=False, stop=True, skip_group_check=True)
                m_tiles[j] = None

            embT_sb = keepp.tile([128, RPC], F16, tag="embT")
            cc_in, cc_out = [], []
            for half in range(2):
                cc_in.append(dramp.tile([64, GPC], F16, tag=f"cc_in{half}",
                                        name=f"cc_in{half}"))
                cc_out.append(dramp.tile([NCORES * 64, GPC], F16,
                                         tag=f"cc_out{half}",
                                         name=f"cc_out{half}"))

            def emit_gather(half):
                rows = slice(64 * half, 64 * half + 64)
                nc.scalar.activation(embT_sb[rows, :], embps[rows, :], AF.Tanh)
                msum = workp.tile([64, GPC], F32, tag="msum",
                                  name=f"msum{half}")
                nc.vector.tensor_reduce(
                    msum[:],
                    embT_sb[rows, :].rearrange("p (g a) -> p g a", a=A),
                    axis=mybir.AxisListType.X, op=OP.add)
                msumh = workp.tile([64, GPC], F16, tag="msumh",
                                   name=f"msumh{half}")
                nc.scalar.copy(msumh[:], msum[:])
                nc.sync.dma_start(cc_in[half][:], msumh[:])
                nc.gpsimd.collective_compute(
                    "AllGather", OP.bypass,
                    replica_groups=[list(range(NCORES))],
                    ins=[cc_in[half].opt()], outs=[cc_out[half].opt()])

            for j in range(NJ + SEL_LAG):
                if j < NJ:
                    emit_hyper(j)
                if j >= SEL_LAG:
                    emit_selector(j - SEL_LAG)
                if j - SEL_LAG == 31:
                    emit_gather(0)
            emit_gather(1)
            if DEBUG:
                nc.sync.dma_start(dbg_hT[:], hT_sb[:])
                nc.sync.dma_start(dbg_emb[:], embT_sb[:])

            # ---- phase 2a (overlaps collective): value MLP + y1a ----
            def chsl(t, ch):
                return t[:, ch * 128:(ch + 1) * 128]

            v1ps = wpsp.tile([128, RPC], F32, tag="wps", name="v1ps")
            for ch in range(8):
                nc.tensor.matmul(chsl(v1ps, ch), vw1_sb[:], chsl(embT_sb, ch),
                                 start=True, stop=True)
            v1T = workp.tile([128, RPC], F16, tag="m", name="v1T")
            nc.scalar.activation(v1T[:], v1ps[:], AF.Relu, bias=vb1_sb[:])

            vps = wpsp.tile([128, RPC], F32, tag="wps", name="vps")
            for ch in range(8):
                nc.tensor.matmul(chsl(vps, ch), onescol_sb[:], vb2r_sb[:],
                                 start=True, stop=False)
                nc.tensor.matmul(chsl(vps, ch), chsl(v1T, ch), vw2_sb[:],
                                 start=False, stop=True)
            vals_sb = keepp.tile([128, RPC], F16, tag="vals")
            nc.scalar.activation(vals_sb[:], vps[:], AF.Relu)

            y1ps = embpsp.tile([128, RPC], F32, tag="embps", name="y1ps")

            # P_w tiles zeroed on gpsimd during the collective
            P_w = []
            for ch in range(8):
                pw = workp.tile([128, GPC], F16, tag=f"P_w{ch}",
                                name=f"P_w_{ch}")
                nc.gpsimd.memset(pw[:], 0.0)
                P_w.append(pw)

            meanT_full = keepp.tile([128, B], F16, tag="meanTf")
            for half in range(2):
                rows = slice(64 * half, 64 * half + 64)
                nc.sync.dma_start(
                    meanT_full[rows, :].rearrange("p (c g) -> p c g",
                                                  c=NCORES),
                    cc_out[half][:].rearrange("(c p) g -> p c g", c=NCORES))

            # ---- phase 2b: attention + weighted output ----
            if DEBUG:
                nc.sync.dma_start(dbg_mean2[:], meanT_full[:])
            for ch in range(8):
                mcol = 128 * (ch % 4)
                nc.tensor.matmul(chsl(y1ps, ch), aw1a_sb[:], chsl(embT_sb, ch),
                                 start=True, stop=False)
                nc.tensor.matmul(chsl(y1ps, ch), aw1b_sb[:],
                                 meanT_full[:, mcol:mcol + 128],
                                 start=False, stop=True)
            y1T = workp.tile([128, RPC], F16, tag="m", name="y1T")
            nc.scalar.activation(y1T[:], y1ps[:], AF.Relu, bias=ab1_sb[:])
            if DEBUG:
                nc.sync.dma_start(dbg_y1T[:], y1T[:])

            scps_t = wpsp.tile([128, RPC], F32, tag="wps", name="scps")
            so_t = wpsp.tile([128, RPC], F32, tag="wps", name="so")
            y2ps = wpsp.tile([128, RPC], F32, tag="wps", name="y2ps")
            for ch in range(8):
                nc.tensor.matmul(chsl(y2ps, ch), aw2_sb[:], chsl(y1T, ch),
                                 start=True, stop=True)
            y2T = workp.tile([128, RPC], F16, tag="m", name="y2T")
            nc.scalar.activation(y2T[:], y2ps[:], AF.Relu, bias=ab2_sb[:])
            for ch in range(8):
                nc.tensor.matmul(scps_t[:, ch:ch + 1], chsl(y2T, ch),
                                 aw3_sb[:], start=True, stop=True)
            exp_sb = workp.tile([128, 8], F32, tag="exp_sb")
            nc.scalar.activation(exp_sb[:], scps_t[:, 0:8], AF.Exp,
                                 bias=ab3_sb[:])
            if DEBUG:
                nc.sync.dma_start(dbg_mean[:], meanT_full[:])
                nc.sync.dma_start(dbg_exp[:], exp_sb[:])
            for ch in range(8):
                nc.gpsimd.tensor_scalar_mul(P_w[ch][:, ch * 8:(ch + 1) * 8],
                                            pmask_sb[:], exp_sb[:, ch:ch + 1])
                nc.tensor.matmul(so_t[0:GPC, 0:128], P_w[ch][:],
                                 chsl(vals_sb, ch),
                                 start=(ch == 0), stop=(ch == 7))
            for ch in range(8):
                nc.tensor.matmul(so_t[0:GPC, 128:129], P_w[ch][:],
                                 ones128_sb[:], start=(ch == 0), stop=(ch == 7))

            inv_S = workp.tile([GPC, 1], F32, tag="inv_S")
            nc.vector.reciprocal(inv_S[:], so_t[0:GPC, 128:129])
            out_sb = workp.tile([GPC, H], F32, tag="out_sb")
            nc.vector.tensor_scalar_mul(out_sb[:], so_t[0:GPC, 0:128],
                                        inv_S[:])
            nc.sync.dma_start(out_d[:], out_sb[:])

    nc.compile()
    return nc


def _prep_inputs(obs, latent, hw1, hb1, hw2, hb2, vw1, vb1, vw2, vb2,
                 aw1, ab1, aw2, ab2, aw3, ab3):
    f = np.float32
    fh = lambda a: np.ascontiguousarray(np.asarray(a, f).astype(NPF16))
    col = lambda a: np.ascontiguousarray(np.asarray(a, f).reshape(-1, 1))

    x_full = np.concatenate(
        [np.tile(obs, (A, 1)), latent.reshape(-1, DO)], axis=1).astype(f)

    # hw2B[k, 128j + 64s + d] = hw2[k, 128d + 2j + s]
    hw2B = np.asarray(hw2, f).reshape(H, D, NJ, 2).transpose(0, 2, 3, 1) \
        .reshape(H, D * H)
    # b2B[64s + d, j] = hb2[128d + 2j + s]
    b2B = np.asarray(hb2, f).reshape(D, NJ, 2).transpose(2, 0, 1) \
        .reshape(128, NJ)
    # corr term only for DVE (max-trick) j's
    bcorr = np.asarray(hb2, f).reshape(D, H).copy()
    for j in ACT_J:
        bcorr[:, 2 * j] = 0.0
        bcorr[:, 2 * j + 1] = 0.0
    selpad = np.zeros((128, 126), NPF16)
    for p in range(128):
        selpad[p, 62 + p // 64] = 1.0
    pmask = np.zeros((128, 8), NPF16)
    for r in range(128):
        pmask[r, r // 16] = 1.0

    shared = dict(
        hw1=fh(hw1), hb1=col(hb1),
        hw2B=fh(hw2B), b2B=np.ascontiguousarray(b2B),
        negb2B=np.ascontiguousarray(-b2B), bcorr=fh(bcorr),
        selpad=selpad,
        vw1=fh(vw1), vb1=col(vb1), vw2=fh(vw2),
        vb2row=fh(np.asarray(vb2).reshape(1, H)),
        onescol=np.ones((1, H), NPF16), ones128=np.ones((H, 1), NPF16),
        aw1a=fh(np.asarray(aw1)[:H]),
        aw1b=fh(np.asarray(aw1)[H:] / A),
        ab1=col(ab1), aw2=fh(aw2), ab2=col(ab2),
        aw3=fh(np.asarray(aw3).reshape(H, 1)),
        ab3=np.full((128, 1), np.float32(np.asarray(ab3).reshape(())), f),
        pmask=pmask,
    )
    in_maps = []
    for c in range(NCORES):
        xcT = x_full[c * RPC:(c + 1) * RPC].T  # [64, 1024]
        m = dict(shared)
        m["xxT"] = np.ascontiguousarray(
            np.concatenate([xcT, xcT], axis=0)).astype(NPF16)
        in_maps.append(m)
    return in_maps


def kernel(**inputs):
    obs = np.asarray(inputs["obs"], np.float32)
    latent = np.asarray(inputs["obs_intention_latent"], np.float32)
    in_maps = _prep_inputs(
        obs, latent, inputs["hw1"], inputs["hb1"], inputs["hw2"], inputs["hb2"],
        inputs["vw1"], inputs["vb1"], inputs["vw2"], inputs["vb2"],
        inputs["aw1"], inputs["ab1"], inputs["aw2"], inputs["ab2"],
        inputs["aw3"], inputs["ab3"])
    if "nc" not in _CACHE:
        _CACHE["nc"] = _build()
    res = run_bass_kernel_spmd(_CACHE["nc"], in_maps, list(range(NCORES)))
    _CACHE["res"] = res
    out = np.empty((B, H), np.float32)
    for c in range(NCORES):
        out[c * GPC:(c + 1) * GPC] = res.results[c]["out"]
    return out


if __name__ == "__main__":
    import reference
    inputs = reference.setup_inputs()
    inputs = {k: np.asarray(v) for k, v in inputs.items()}
    got = kernel(**inputs)
    exp = np.asarray(reference.reference(**reference.setup_inputs()))
    print("Relative error:", np.abs(got - exp).max() / (np.abs(exp).max() + 1e-9))
